# revision 8
# baseline (speedup 1.0000x reference)
"""Trainium2 Bass kernel for nn_Crop (per-row random crop of audio).

Reference semantics:
    out[i, j] = audio[i, j]             for j <  starts[i]
    out[i, j] = audio[i, j + CROP_NUM]  for j >= starts[i]

Strategy (pure data parallel, 16 rows per core across 8 cores):
out[i] is an elementwise select between the row read at offset 0
(identity) and the row read at offset CROP (shifted), keyed on
global position < starts[i].  Per row: two plain strided DMA loads
into [116, 2048] SBUF tiles (no indirection — the shifted view is just
audio[i, CROP : CROP + 116*2048] reshaped), one tensor_scalar is_lt
building the mask from a precomputed global-position iota against the
row's start (per-partition scalar), one copy_predicated blending the
identity values over the shifted tile, one store.  Handles every lane
including the straddling block exactly — no boundary fixup, no host
splice.  The last row's shifted load is split to stay in bounds; other
rows over-read into the next row (harmless, host-trimmed).

A single ExternalOutput tensor is load-bearing: each extra output costs
~85ms of axon-relay dispatch overhead per call (measured), dwarfing the
~0.3ms device time.  Device HBM traffic/row = read ~2x OUT_LEN + write
OUT_LEN; at ~200GB/s per-core DMA this is ~0.25ms, invisible under the
~41ms dispatch floor.  Inputs are zero-copy views of the caller's audio.
"""

import numpy as np

import concourse.bacc as bacc
import concourse.mybir as mybir
from concourse import bass_utils
from concourse.tile import TileContext

# Problem constants (hardcoded per harness contract).
B = 128
L = 262144
CROP = 26214
OUT_LEN = L - CROP  # 235930
N_CORES = 8
R = B // N_CORES  # 16 rows per core

W = 2048                      # block width; L == 128 * W
N_BLK = OUT_LEN // W + 1      # 116 blocks cover one output row
N_FULL = OUT_LEN // W         # 115 full blocks
TAIL = OUT_LEN - N_FULL * W   # 410
PADW = N_BLK * W              # out row padded to 237568 so every row
                              # store is one 8KB-aligned [116, W] DMA

_programs = {}


def _build_program(reps: int = 1):
    """Build the single SPMD Bass/Tile program (shared by all 8 cores).

    reps > 1 wraps the body in an on-device For_i loop for benchmarking
    (isolates device time from the axon dispatch overhead).
    """
    if reps in _programs:
        return _programs[reps]
    nc = bacc.Bacc("TRN2", target_bir_lowering=False, debug=False)

    audio = nc.dram_tensor(
        "audio", [R * L], mybir.dt.float32, kind="ExternalInput"
    ).ap()
    s_rep = nc.dram_tensor(
        "s_rep", [N_BLK, R], mybir.dt.float32, kind="ExternalInput"
    ).ap()
    glob_pos = nc.dram_tensor(
        "glob_pos", [N_BLK, W], mybir.dt.float32, kind="ExternalInput"
    ).ap()
    out = nc.dram_tensor(
        "out", [R, PADW], mybir.dt.float32, kind="ExternalOutput"
    ).ap()

    with TileContext(nc) as tc:
        with (
            tc.tile_pool(name="consts", bufs=1) as consts,
            tc.tile_pool(name="work", bufs=4) as work,
        ):
            s_rep_sb = consts.tile([N_BLK, R], mybir.dt.float32)
            glob_pos_sb = consts.tile([N_BLK, W], mybir.dt.float32)
            nc.sync.dma_start(out=s_rep_sb[:], in_=s_rep[:])
            nc.sync.dma_start(out=glob_pos_sb[:], in_=glob_pos[:])

            def body():
                for i in range(R):
                    t_sh = work.tile([N_BLK, W], mybir.dt.float32, tag="sh")
                    t_id = work.tile([N_BLK, W], mybir.dt.float32, tag="id")
                    mask = work.tile([N_BLK, W], mybir.dt.uint8, tag="mask")
                    # shifted view: audio[i*L + CROP + k*W + j]
                    base = i * L + CROP
                    if i < R - 1:
                        # over-reads 1638 elems into row i+1 (lane 115,
                        # cols >= 410): harmless, host-trimmed
                        src = audio[base : base + N_BLK * W].rearrange(
                            "(p w) -> p w", w=W
                        )
                        nc.sync.dma_start(out=t_sh[:], in_=src)
                    else:
                        # last row: stay inside the input tensor
                        src = audio[base : base + N_FULL * W].rearrange(
                            "(p w) -> p w", w=W
                        )
                        nc.sync.dma_start(out=t_sh[:N_FULL, :], in_=src)
                        tail = audio[
                            base + N_FULL * W : base + N_FULL * W + TAIL
                        ].rearrange("(p w) -> p w", w=TAIL)
                        nc.sync.dma_start(
                            out=t_sh[N_FULL : N_FULL + 1, :TAIL], in_=tail
                        )
                        # lane 115 cols >= 410 are never selected by a
                        # valid mask, but keep them defined
                        nc.sync.dma_start(
                            out=t_sh[N_FULL : N_FULL + 1, TAIL:],
                            in_=audio[i * L : i * L + W - TAIL].rearrange(
                                "(p w) -> p w", w=W - TAIL
                            ),
                        )
                    # identity view: audio[i*L + k*W + j]
                    src_id = audio[i * L : i * L + N_BLK * W].rearrange(
                        "(p w) -> p w", w=W
                    )
                    nc.scalar.dma_start(out=t_id[:], in_=src_id)
                    # mask = (k*W + j) < starts[i]  -> take identity there
                    nc.vector.tensor_scalar(
                        mask[:], glob_pos_sb[:], s_rep_sb[:, i : i + 1],
                        None, mybir.AluOpType.is_lt,
                    )
                    nc.vector.copy_predicated(t_sh[:], mask[:], t_id[:])
                    dst = out[i, :].rearrange("(p w) -> p w", w=W)
                    nc.gpsimd.dma_start(out=dst, in_=t_sh[:])

            if reps == 1:
                body()
            else:
                with tc.For_i(0, reps, 1):
                    body()

    nc.compile()
    _programs[reps] = nc
    return nc


_GLOB_POS = None


def _host_inputs(audio: np.ndarray, starts: np.ndarray):
    """Shard per core: audio slices are zero-copy views; consts are tiny."""
    global _GLOB_POS
    audio = np.ascontiguousarray(audio, dtype=np.float32)
    starts = np.asarray(starts, dtype=np.int32)

    if _GLOB_POS is None:
        _GLOB_POS = (
            np.arange(N_BLK, dtype=np.float32)[:, None] * W
            + np.arange(W, dtype=np.float32)[None, :]
        )  # [116, 2048], exact in f32 (max 237567 < 2^24)

    in_maps = []
    for c in range(N_CORES):
        rows = slice(c * R, (c + 1) * R)
        s_rep = np.broadcast_to(
            starts[rows].astype(np.float32)[None, :], (N_BLK, R)
        ).copy()
        in_maps.append(
            {
                "audio": audio[rows].reshape(-1),  # zero-copy view
                "s_rep": s_rep,
                "glob_pos": _GLOB_POS,
            }
        )
    return in_maps


def _unshard(results):
    out = np.empty((B, OUT_LEN), dtype=np.float32)
    for c in range(N_CORES):
        out[c * R : (c + 1) * R] = results[c]["out"][:, :OUT_LEN]
    return out


def kernel(audio: np.ndarray, starts: np.ndarray) -> np.ndarray:
    nc = _build_program()
    in_maps = _host_inputs(audio, starts)
    res = bass_utils.run_bass_kernel_spmd(
        nc, in_maps, core_ids=list(range(N_CORES))
    )
    kernel.last_results = res
    return _unshard(res.results)
